# revision 1
# baseline (speedup 1.0000x reference)
"""GAT message-passing kernel for 8 trn2 NeuronCores.

Math (reference):
    Wx = x @ W;  s1 = Wx@a1/sqrt(2D);  s2 = Wx@a2/sqrt(2D)   (per t)
    weight = softmax_m(lrelu(s1[m] + s2[n]));  agg = lrelu(weight @ Wx)
    out = x - agg

Key identities:
  * lrelu(v) = max(v, 0.01v) and exp monotone =>
        exp(lrelu(s1+s2)) = max(exp(s1+s2), exp(0.01(s1+s2)))
  * softmax is invariant to per-n rescaling; dividing by exp(s2[n]):
        E~[m,n] = max(E1[m], F1[m] * r[n])
    with E1=exp(s1), F1=exp(0.01 s1), r=exp(-0.99 s2) - O(N) exps only.
    The O(N^2) score tile is ONE fused DVE tensor_scalar:
        (r_b mult F1col) max E1col.
  * softmax denominator folded into the aggregation matmul as a ones
    column appended to Wx.
  * out = x - lrelu(agg) = min(x - agg, x - 0.01*agg).

Sharding: 8 cores = 4 t-slices x 2 N-halves; each core aggregates over all
4096 source nodes for its own (t, 2048 dest nodes).
"""

import sys

if "/opt/trn_rl_repo" not in sys.path:
    sys.path.insert(0, "/opt/trn_rl_repo")

import numpy as np

N, T, D = 4096, 4, 128
P = 128
HALF = N // 2            # 2048 dest nodes per core
MT = N // P              # 32 m tiles
NT = HALF // P           # 16 own n tiles
NQ = HALF // 512         # 4 n chunks of 512
SCALE_INV = 1.0 / 16.0   # 1/sqrt(2*128)

# packed input column offsets: [params | xT | xn]
XCH = N // 4             # 1024
C_PRM = 0
C_XT = 2 * D + 2         # 258
C_XN = C_XT + N          # 4354
C_END = C_XN + HALF      # 6402

_CACHE = {}


def _build():
    import concourse.mybir as mybir
    from concourse import bacc
    from concourse.tile import TileContext

    f32 = mybir.dt.float32
    bf16 = mybir.dt.bfloat16
    Alu = mybir.AluOpType
    Act = mybir.ActivationFunctionType

    nc = bacc.Bacc()
    xin = nc.declare_dram_parameter("xin", [P, C_END], f32, isOutput=False)
    out = nc.declare_dram_parameter("out", [HALF, D], f32, isOutput=True)

    with TileContext(nc) as tc:
        with (
            tc.tile_pool(name="const", bufs=1) as cpool,
            tc.tile_pool(name="epool", bufs=12) as epool,
            tc.tile_pool(name="fpool", bufs=4) as fpool,
            tc.tile_pool(name="opool", bufs=4) as opool,
        ):
            # ---- input DMAs: 4 xT chunks (so projection starts early),
            # then the rest (xn + params) ----
            # chunk 0 carries the params + first quarter of xT
            px0 = cpool.tile([P, C_XT + XCH], f32)
            nc.sync.dma_start(px0[:, :], xin[:, 0 : C_XT + XCH])
            prm = px0[:, 0:C_XT]
            xts = [px0[:, C_XT : C_XT + XCH]]
            for ch in range(1, 4):
                xt_c = cpool.tile([P, XCH], f32, name=f"xt{ch}", tag=f"xt{ch}")
                nc.sync.dma_start(
                    xt_c[:, :], xin[:, C_XT + ch * XCH : C_XT + (ch + 1) * XCH]
                )
                xts.append(xt_c)
            xn_sb = cpool.tile([P, HALF], f32)
            nc.gpsimd.dma_start(xn_sb[:, :], xin[:, C_XN:C_END])
            Wm_sb = prm[:, 0:D]
            WT_sb = prm[:, D : 2 * D]
            av_sb = prm[:, 2 * D : 2 * D + 2]

            wx = cpool.tile([P, MT * (D + 1)], bf16)
            E1 = cpool.tile([P, MT], f32)
            F1 = cpool.tile([P, MT], f32)
            r_b = cpool.tile([P, HALF], bf16)

            with tc.tile_pool(name="ppsum", bufs=2, space="PSUM") as ppool:
                # ---- wproj = [W | w1 | w2] ----
                wproj = cpool.tile([P, D + 2], f32)
                nc.scalar.activation(wproj[:, :D], Wm_sb, Act.Copy)
                w_ps = ppool.tile([P, 2], f32, tag="ps", name="w_ps")
                nc.tensor.matmul(w_ps[:, :], WT_sb, av_sb, start=True, stop=True)
                nc.scalar.activation(
                    wproj[:, D : D + 2], w_ps[:, :], Act.Copy, scale=SCALE_INV
                )
                ones_col = cpool.tile([P, 1], f32)
                nc.scalar.activation(
                    ones_col[:, :], prm[:, 0:1], Act.Copy, scale=0.0, bias=1.0
                )

                # ---- r_b[p, n] = exp(-0.99 * s2[n]) for all p, via one
                # rank-1 stationary (w2 broadcast along free dim): a single
                # full-shape matmul per 512-chunk computes s2 replicated
                # across all 128 partitions; exp lands straight in r_b ----
                w2b = cpool.tile([P, P], f32)
                nc.vector.tensor_scalar(
                    w2b[:, :], Wm_sb, 0.0, wproj[:, D + 1 : D + 2],
                    Alu.mult, Alu.add,
                )
                for q in range(NQ):
                    rb_ps = ppool.tile([P, 512], f32, tag="ps", name="rb_ps")
                    nc.tensor.matmul(
                        rb_ps[:, :],
                        w2b[:, :],
                        xts[q // 2][:, (q % 2) * 512 : (q % 2) * 512 + 512],
                        start=True,
                        stop=True,
                    )
                    nc.scalar.activation(
                        r_b[:, q * 512 : (q + 1) * 512],
                        rb_ps[:, :],
                        Act.Exp,
                        scale=-0.99,
                    )

                # ---- projection: wx = [Wx(bf16) | 1] per mt, E1/F1 ----
                for mt in range(MT):
                    p_ps = ppool.tile(
                        [P, D + 2], f32, tag="pp", name="p_ps", bufs=6
                    )
                    nc.tensor.matmul(
                        p_ps[:, :],
                        xts[mt // 8][:, (mt % 8) * P : (mt % 8) * P + P],
                        wproj[:, :],
                        start=True,
                        stop=True,
                    )
                    base = mt * (D + 1)
                    nc.vector.tensor_copy(wx[:, base : base + D], p_ps[:, :D])
                    nc.scalar.activation(
                        wx[:, base + D : base + D + 1], ones_col[:, :], Act.Copy
                    )
                    nc.scalar.activation(
                        E1[:, mt : mt + 1], p_ps[:, D : D + 1], Act.Exp
                    )
                    nc.scalar.activation(
                        F1[:, mt : mt + 1], p_ps[:, D : D + 1], Act.Exp, scale=0.01
                    )

            # ---- main: score tiles + aggregation (double-buffered acc) ----
            with tc.tile_pool(name="mpsum", bufs=2, space="PSUM") as mpool:
                def finalize(q, acc, o_q):
                    for j in range(4):
                        nt = q * 4 + j
                        rz = fpool.tile([P, 1], f32, tag="rz", name="rz")
                        nc.vector.reciprocal(rz[:, :], acc[j][:, D : D + 1])
                        # lrelu(agg) in one ACT op: Lrelu(rz * numer), slope 0.01
                        lr = fpool.tile([P, D], f32, tag="lr", name="lr")
                        nc.scalar.activation(
                            lr[:, :],
                            acc[j][:, :D],
                            Act.Lrelu,
                            scale=rz[:, :],
                            alpha=0.01,
                        )
                        nc.vector.tensor_tensor(
                            o_q[:, j * D : (j + 1) * D],
                            xn_sb[:, nt * D : (nt + 1) * D],
                            lr[:, :],
                            Alu.subtract,
                        )
                    out_view = out[q * 512 : (q + 1) * 512, :].rearrange(
                        "(j p) d -> p j d", p=P
                    )
                    nc.sync.dma_start(
                        out_view, o_q.rearrange("p (j d) -> p j d", j=4)
                    )

                pending = None
                for q in range(NQ):
                    acc = [
                        mpool.tile([P, D + 1], f32, tag=f"acc{j}", name=f"acc{j}")
                        for j in range(4)
                    ]
                    o_q = opool.tile([P, 4 * D], f32, name="o_q")
                    for mt in range(MT):
                        et = epool.tile([P, 512], bf16, name="et")
                        nc.vector.tensor_scalar(
                            et[:, :],
                            r_b[:, q * 512 : (q + 1) * 512],
                            F1[:, mt : mt + 1],
                            E1[:, mt : mt + 1],
                            Alu.mult,
                            Alu.max,
                        )
                        if mt == 6 and pending is not None:
                            finalize(*pending)
                            pending = None
                        base = mt * (D + 1)
                        for j in range(4):
                            nc.tensor.matmul(
                                acc[j][:, :],
                                et[:, j * P : (j + 1) * P],
                                wx[:, base : base + D + 1],
                                start=(mt == 0),
                                stop=(mt == MT - 1),
                            )
                    pending = (q, acc, o_q)
                finalize(*pending)

    nc.compile()
    return nc


def _prep_inputs(x, W, a1, a2):
    """Per-core packed input. Core c: t = c//2, n-half h = c%2.

    xT is host-rotated so the core's own 2048 dest columns come first
    (a rotation does not change a sum over all source nodes).
    """
    x = np.asarray(x, dtype=np.float32)
    W = np.ascontiguousarray(np.asarray(W, dtype=np.float32))
    WT = np.ascontiguousarray(W.T)
    av = np.ascontiguousarray(
        np.stack([np.asarray(a1, np.float32), np.asarray(a2, np.float32)], axis=1)
    )
    in_maps = []
    for c in range(8):
        t, h = c // 2, c % 2
        xt = x[:, t, :].T  # [D, N]
        if h == 1:
            xt = np.concatenate([xt[:, HALF:], xt[:, :HALF]], axis=1)
        xn = x[h * HALF : (h + 1) * HALF, t, :]  # [2048, 128]
        xn_packed = xn.reshape(NT, P, D).transpose(1, 0, 2).reshape(P, NT * D)
        xin = np.concatenate([W, WT, av, xt, xn_packed], axis=1)
        in_maps.append({"xin": np.ascontiguousarray(xin)})
    return in_maps


def _run(x, W, a1, a2, trace=False):
    from concourse.bass_utils import run_bass_kernel_spmd

    key = "nc"
    if key not in _CACHE:
        _CACHE[key] = _build()
    nc = _CACHE[key]
    in_maps = _prep_inputs(x, W, a1, a2)
    res = run_bass_kernel_spmd(nc, in_maps, list(range(8)), trace=trace)
    out_full = np.empty((N, T, D), dtype=np.float32)
    for c in range(8):
        t, h = c // 2, c % 2
        out_full[h * HALF : (h + 1) * HALF, t, :] = res.results[c]["out"]
    return out_full, res


def kernel(x, W, a1, a2):
    out, _ = _run(x, W, a1, a2, trace=False)
    return out



# revision 3
# speedup vs baseline: 1.0023x; 1.0023x over previous
"""GAT message-passing kernel for 8 trn2 NeuronCores (v2).

Math (reference):
    Wx = x @ W;  s1 = Wx@a1/sqrt(2D);  s2 = Wx@a2/sqrt(2D)   (per t)
    weight = softmax_m(lrelu(s1[m] + s2[n]));  agg = lrelu(weight @ Wx)
    out = x - agg

Key identities:
  * exp(lrelu(s1+s2)) = max(exp(s1+s2), exp(0.01(s1+s2))); dividing the
    softmax row by exp(s2[n]) (softmax-invariant):
        et_true[m,n] = F1[m] * max(q[m], r[n])
    with q = exp(0.99*s1), F1 = exp(0.01*s1), r = exp(-0.99*s2).
  * F1 folds into the matmul moving operand [F1*Wx | F1], so the score
    tile is ONE single-op DVE tensor_scalar max(r_b, q[m]) (4x_2P mode)
    and the softmax denominator is the accumulated F1 column (exact).
  * out = x - lrelu(num/Z) = x - Lrelu(rz*num) via ACT per-partition scale.

Sharding: 8 cores = 4 t-slices x 2 N-halves; each core aggregates over all
4096 source nodes for its own (t, 2048 dest nodes).

Loop: mt-outer, fully pipelined with the input DMA. Per source tile mt:
projection matmul -> s1 stage -> F1 exp -> evac (F1 fold) -> one
[128,2048] et tile -> 16 accumulation matmuls (4 q x 4 j chunks).
PSUM: 2 banks scratch (proj/r_b) + 6 banks for 16 accumulators packed
three-per-bank.
"""

import sys

if "/opt/trn_rl_repo" not in sys.path:
    sys.path.insert(0, "/opt/trn_rl_repo")

import numpy as np

N, T, D = 4096, 4, 128
P = 128
HALF = N // 2            # 2048 dest nodes per core
MT = N // P              # 32 source tiles
NQ = HALF // 512         # 4 dest chunks of 512
DC = D + 1               # moving operand cols: [F1*Wx | F1]
SCALE_INV = 1.0 / 16.0   # 1/sqrt(2*128)

_CACHE = {}


def _build():
    import concourse.mybir as mybir
    from concourse import bacc
    from concourse.tile import TileContext

    f32 = mybir.dt.float32
    bf16 = mybir.dt.bfloat16
    Alu = mybir.AluOpType
    Act = mybir.ActivationFunctionType

    nc = bacc.Bacc()
    wp = nc.declare_dram_parameter("wp", [P, DC], bf16, isOutput=False)
    w2 = nc.declare_dram_parameter("w2", [P, 1], f32, isOutput=False)
    xt = nc.declare_dram_parameter("xt", [P, N], bf16, isOutput=False)
    xn = nc.declare_dram_parameter("xn", [P, HALF], f32, isOutput=False)
    out = nc.declare_dram_parameter("out", [HALF, D], f32, isOutput=True)

    with TileContext(nc) as tc:
        with (
            tc.tile_pool(name="const", bufs=1) as cpool,
            tc.tile_pool(name="epool", bufs=3) as epool,
            tc.tile_pool(name="fpool", bufs=4) as fpool,
            tc.tile_pool(name="opool", bufs=2) as opool,
        ):
            # ---- input DMAs: params first, then xT quarters, xn last ----
            wp_sb = cpool.tile([P, DC], bf16)
            w2_sb = cpool.tile([P, 1], f32)
            nc.sync.dma_start(wp_sb[:, :], wp[:, :])
            nc.sync.dma_start(w2_sb[:, :], w2[:, :])
            xt_sb = cpool.tile([P, N], bf16)
            for ch in range(4):
                nc.sync.dma_start(
                    xt_sb[:, ch * 1024 : (ch + 1) * 1024],
                    xt[:, ch * 1024 : (ch + 1) * 1024],
                )
            xn_sb = cpool.tile([P, HALF], f32)
            nc.gpsimd.dma_start(xn_sb[:, :], xn[:, :])

            # persistent SBUF state
            wxm = cpool.tile([P, MT * DC], bf16)     # [F1*Wx | F1] per mt
            r_b = cpool.tile([P, HALF], bf16)        # exp(-0.99 s2[n]) rows
            s1_all = cpool.tile([P, MT], f32)
            q_all = cpool.tile([P, MT], f32)         # exp(0.99 s1)
            f1_all = cpool.tile([P, MT], f32)        # exp(0.01 s1)

            with (
                tc.tile_pool(name="spsum", bufs=2, space="PSUM") as spool,
                tc.tile_pool(name="mpsum", bufs=1, space="PSUM") as mpool,
            ):
                # ---- w2b[k,m] = w2[k] broadcast: rank-1 stationary ----
                w2b = cpool.tile([P, P], bf16)
                nc.vector.tensor_scalar(
                    w2b[:, :], wp_sb[:, 0:P], 0.0, w2_sb[:, :],
                    Alu.mult, Alu.add,
                )
                # ---- r_b = exp(-0.99 * s2) via replicated-row matmuls ----
                for c in range(4):
                    rb_ps = spool.tile([P, 512], f32, tag="sc", name="rb_ps")
                    nc.tensor.matmul(
                        rb_ps[:, :], w2b[:, :],
                        xt_sb[:, c * 512 : (c + 1) * 512],
                        start=True, stop=True,
                    )
                    nc.scalar.activation(
                        r_b[:, c * 512 : (c + 1) * 512], rb_ps[:, :],
                        Act.Exp, scale=-0.99,
                    )

                # ---- 16 accumulators packed 3-per-bank: 5 trios + 1 ----
                trio = [
                    mpool.tile([P, 3 * DC], f32, tag=f"tr{t}", name=f"tr{t}")
                    for t in range(5)
                ] + [mpool.tile([P, DC], f32, tag="tr5", name="tr5")]

                def acc_view(q, j):
                    idx = q * 4 + j
                    t, s = idx // 3, idx % 3
                    return trio[t][:, s * DC : (s + 1) * DC]

                # ---- main loop: mt-outer, proj fused in ----
                for mt in range(MT):
                    p_ps = spool.tile([P, 512], f32, tag="sc", name="p_ps")
                    nc.tensor.matmul(
                        p_ps[:, 0:DC],
                        xt_sb[:, mt * P : (mt + 1) * P],
                        wp_sb[:, :],
                        start=True, stop=True,
                    )
                    nc.vector.tensor_copy(
                        s1_all[:, mt : mt + 1], p_ps[:, D : D + 1]
                    )
                    nc.scalar.activation(
                        f1_all[:, mt : mt + 1], p_ps[:, D : D + 1],
                        Act.Exp, scale=0.01,
                    )
                    nc.vector.tensor_copy(
                        wxm[:, mt * DC + D : (mt + 1) * DC],
                        f1_all[:, mt : mt + 1],
                    )
                    nc.vector.tensor_scalar(
                        wxm[:, mt * DC : mt * DC + D],
                        p_ps[:, 0:D],
                        f1_all[:, mt : mt + 1],
                        None,
                        Alu.mult,
                    )
                    if mt % 4 == 3:
                        sl = slice(mt - 3, mt + 1)
                        nc.scalar.activation(
                            q_all[:, sl], s1_all[:, sl], Act.Exp, scale=0.99
                        )
                    et = epool.tile([P, HALF], bf16, name="et")
                    nc.vector.tensor_scalar(
                        et[:, :], r_b[:, :], q_all[:, mt : mt + 1],
                        None, Alu.max,
                    )
                    wv = wxm[:, mt * DC : (mt + 1) * DC]
                    for q in range(NQ):
                        for j in range(4):
                            nc.tensor.matmul(
                                acc_view(q, j),
                                et[:, q * 512 + j * P : q * 512 + (j + 1) * P],
                                wv,
                                start=(mt == 0),
                                stop=(mt == MT - 1),
                            )

                # ---- finalize ----
                for q in range(NQ):
                    lr = fpool.tile([P, 512], f32, tag="lr", name="lr")
                    for j in range(4):
                        av = acc_view(q, j)
                        rz = fpool.tile([P, 1], f32, tag="rz", name="rz")
                        nc.vector.reciprocal(rz[:, :], av[:, D : D + 1])
                        nc.scalar.activation(
                            lr[:, j * P : (j + 1) * P],
                            av[:, :D],
                            Act.Lrelu,
                            scale=rz[:, :],
                            alpha=0.01,
                        )
                    o_q = opool.tile([P, 512], f32, name="o_q")
                    nc.vector.tensor_tensor(
                        o_q[:, :], xn_sb[:, q * 512 : (q + 1) * 512],
                        lr[:, :], Alu.subtract,
                    )
                    out_view = out[q * 512 : (q + 1) * 512, :].rearrange(
                        "(j p) d -> p j d", p=P
                    )
                    nc.sync.dma_start(
                        out_view, o_q.rearrange("p (j d) -> p j d", j=4)
                    )

    nc.compile()
    return nc


def _prep_inputs(x, W, a1, a2):
    """Per-core packed inputs. Core c: t = c//2, n-half h = c%2.

    xT is host-rotated so the core's own 2048 dest columns come first
    (a rotation does not change a sum over all source nodes).
    """
    import ml_dtypes

    x = np.asarray(x, dtype=np.float32)
    W = np.asarray(W, dtype=np.float32)
    w1 = (W @ np.asarray(a1, np.float32)) * SCALE_INV
    w2 = (W @ np.asarray(a2, np.float32)) * SCALE_INV
    wp = np.concatenate([W, w1[:, None]], axis=1).astype(ml_dtypes.bfloat16)
    w2c = np.ascontiguousarray(w2[:, None])
    in_maps = []
    for c in range(8):
        t, h = c // 2, c % 2
        xt = x[:, t, :].T  # [D, N]
        if h == 1:
            xt = np.concatenate([xt[:, HALF:], xt[:, :HALF]], axis=1)
        xn = x[h * HALF : (h + 1) * HALF, t, :]  # [2048, 128]
        xn_packed = (
            xn.reshape(HALF // P, P, D).transpose(1, 0, 2).reshape(P, HALF)
        )
        in_maps.append(
            {
                "wp": np.ascontiguousarray(wp),
                "w2": w2c,
                "xt": np.ascontiguousarray(xt.astype(ml_dtypes.bfloat16)),
                "xn": np.ascontiguousarray(xn_packed),
            }
        )
    return in_maps


def _run(x, W, a1, a2, trace=False):
    from concourse.bass_utils import run_bass_kernel_spmd

    key = "nc"
    if key not in _CACHE:
        _CACHE[key] = _build()
    nc = _CACHE[key]
    in_maps = _prep_inputs(x, W, a1, a2)
    res = run_bass_kernel_spmd(nc, in_maps, list(range(8)), trace=trace)
    out_full = np.empty((N, T, D), dtype=np.float32)
    for c in range(8):
        t, h = c // 2, c % 2
        out_full[h * HALF : (h + 1) * HALF, t, :] = res.results[c]["out"]
    return out_full, res


def kernel(x, W, a1, a2):
    out, _ = _run(x, W, a1, a2, trace=False)
    return out


# revision 6
# speedup vs baseline: 1.2150x; 1.2122x over previous
"""GAT message-passing kernel for 8 trn2 NeuronCores (v2).

Math (reference):
    Wx = x @ W;  s1 = Wx@a1/sqrt(2D);  s2 = Wx@a2/sqrt(2D)   (per t)
    weight = softmax_m(lrelu(s1[m] + s2[n]));  agg = lrelu(weight @ Wx)
    out = x - agg

Key identities:
  * exp(lrelu(s1+s2)) = max(exp(s1+s2), exp(0.01(s1+s2))); dividing the
    softmax row by exp(s2[n]) (softmax-invariant):
        et_true[m,n] = F1[m] * max(q[m], r[n])
    with q = exp(0.99*s1), F1 = exp(0.01*s1), r = exp(-0.99*s2).
  * F1 folds into the matmul moving operand [F1*Wx | F1], so the score
    tile is ONE single-op DVE tensor_scalar max(r_b, q[m]) (4x_2P mode)
    and the softmax denominator is the accumulated F1 column (exact).
  * out = x - lrelu(num/Z) = x - Lrelu(rz*num) via ACT per-partition scale.

Sharding: 8 cores = 4 t-slices x 2 N-halves; each core aggregates over all
4096 source nodes for its own (t, 2048 dest nodes).

Loop: mt-outer, fully pipelined with the input DMA. Per source tile mt:
projection matmul -> s1 stage -> F1 exp -> evac (F1 fold) -> one
[128,2048] et tile -> 16 accumulation matmuls (4 q x 4 j chunks).
PSUM: 2 banks scratch (proj/r_b) + 6 banks for 16 accumulators packed
three-per-bank.
"""

import sys

if "/opt/trn_rl_repo" not in sys.path:
    sys.path.insert(0, "/opt/trn_rl_repo")

import numpy as np

N, T, D = 4096, 4, 128
P = 128
HALF = N // 2            # 2048 dest nodes per core
MT = N // P              # 32 source tiles
NQ = HALF // 512         # 4 dest chunks of 512
DC = D + 1               # moving operand cols: [F1*Wx | F1]
SCALE_INV = 1.0 / 16.0   # 1/sqrt(2*128)

_CACHE = {}


def _build():
    import concourse.mybir as mybir
    from concourse import bacc
    from concourse.tile import TileContext

    f32 = mybir.dt.float32
    bf16 = mybir.dt.bfloat16
    Alu = mybir.AluOpType
    Act = mybir.ActivationFunctionType

    nc = bacc.Bacc()
    wp = nc.declare_dram_parameter("wp", [P, DC], bf16, isOutput=False)
    w2 = nc.declare_dram_parameter("w2", [P, 1], f32, isOutput=False)
    xt = nc.declare_dram_parameter("xt", [P, N], bf16, isOutput=False)
    xn = nc.declare_dram_parameter("xn", [P, HALF], f32, isOutput=False)
    out = nc.declare_dram_parameter("out", [HALF, D], f32, isOutput=True)

    with TileContext(nc) as tc:
        with (
            tc.tile_pool(name="const", bufs=1) as cpool,
            tc.tile_pool(name="epool", bufs=3) as epool,
            tc.tile_pool(name="fpool", bufs=4) as fpool,
            tc.tile_pool(name="opool", bufs=2) as opool,
        ):
            # ---- input DMAs: params first, then xT quarters, xn last ----
            wp_sb = cpool.tile([P, DC], bf16)
            w2_sb = cpool.tile([P, 1], f32)
            nc.sync.dma_start(wp_sb[:, :], wp[:, :])
            nc.sync.dma_start(w2_sb[:, :], w2[:, :])
            xt_sb = cpool.tile([P, N], bf16)
            for ch in range(4):
                nc.sync.dma_start(
                    xt_sb[:, ch * 1024 : (ch + 1) * 1024],
                    xt[:, ch * 1024 : (ch + 1) * 1024],
                )
            xn_sb = cpool.tile([P, HALF], f32)

            # persistent SBUF state
            wxm = cpool.tile([P, MT * DC], bf16)     # [Wx | -1] per mt
            r_b = cpool.tile([P, HALF], bf16)        # exp(-0.99 s2[n]) rows
            q_all = cpool.tile([P, MT], f32)         # exp(0.99 s1)

            with (
                tc.tile_pool(name="spsum", bufs=2, space="PSUM") as spool,
                tc.tile_pool(name="mpsum", bufs=1, space="PSUM") as mpool,
            ):
                # ---- w2b[k,m] = w2[k] broadcast: rank-1 stationary ----
                w2b = cpool.tile([P, P], bf16)
                nc.vector.tensor_scalar(
                    w2b[:, :], wp_sb[:, 0:P], 0.0, w2_sb[:, :],
                    Alu.mult, Alu.add,
                )
                # constant -1 column in every moving-operand slot: the
                # accumulated col 128 is then -Z, so reciprocal gives -1/Z
                # and the finalize is a fused (lr * rz) + xn.
                nc.scalar.activation(
                    wxm.rearrange("p (m c) -> p m c", c=DC)[:, :, D : D + 1],
                    wp_sb[:, 0:MT],
                    Act.Copy, scale=0.0, bias=-1.0,
                )
                # ---- r_b = exp(-0.99 * s2) via replicated-row matmuls ----
                for c in range(4):
                    rb_ps = spool.tile([P, 512], f32, tag="sc", name="rb_ps")
                    nc.tensor.matmul(
                        rb_ps[:, :], w2b[:, :],
                        xt_sb[:, c * 512 : (c + 1) * 512],
                        start=True, stop=True,
                    )
                    nc.scalar.activation(
                        r_b[:, c * 512 : (c + 1) * 512], rb_ps[:, :],
                        Act.Exp, scale=-0.99,
                    )

                # ---- 16 accumulators packed 3-per-bank: 5 trios + 1 ----
                trio = [
                    mpool.tile([P, 3 * DC], f32, tag=f"tr{t}", name=f"tr{t}")
                    for t in range(5)
                ] + [mpool.tile([P, DC], f32, tag="tr5", name="tr5")]

                def acc_view(q, j):
                    idx = q * 4 + j
                    t, s = idx // 3, idx % 3
                    return trio[t][:, s * DC : (s + 1) * DC]

                # ---- main loop: mt-outer, proj fused in ----
                for mt in range(MT):
                    p_ps = spool.tile([P, 512], f32, tag="sc", name="p_ps")
                    nc.tensor.matmul(
                        p_ps[:, 0:DC],
                        xt_sb[:, mt * P : (mt + 1) * P],
                        wp_sb[:, :],
                        start=True, stop=True,
                    )
                    nc.scalar.activation(
                        q_all[:, mt : mt + 1], p_ps[:, D : D + 1],
                        Act.Exp, scale=0.99,
                    )
                    nc.scalar.activation(
                        wxm[:, mt * DC : mt * DC + D], p_ps[:, 0:D], Act.Copy
                    )
                    if mt == 2:
                        nc.gpsimd.dma_start(xn_sb[:, :], xn[:, :])
                    et = epool.tile([P, HALF], bf16, name="et")
                    nc.vector.tensor_scalar(
                        et[:, :], r_b[:, :], q_all[:, mt : mt + 1],
                        None, Alu.max,
                    )
                    wv = wxm[:, mt * DC : (mt + 1) * DC]
                    for q in range(NQ):
                        for j in range(4):
                            nc.tensor.matmul(
                                acc_view(q, j),
                                et[:, q * 512 + j * P : q * 512 + (j + 1) * P],
                                wv,
                                start=(mt == 0),
                                stop=(mt == MT - 1),
                            )

                # ---- finalize: lrelu whole trios, then (lr*(-1/Z)) + xn ----
                lrt = [
                    fpool.tile([P, 3 * DC], f32, tag=f"lrt{t}", name=f"lrt{t}")
                    for t in range(5)
                ] + [fpool.tile([P, DC], f32, tag="lrt5", name="lrt5")]
                for t in range(6):
                    nc.scalar.activation(
                        lrt[t][:, :], trio[t][:, :], Act.Lrelu, alpha=0.01
                    )

                def lr_view(q, j):
                    idx = q * 4 + j
                    t, s = idx // 3, idx % 3
                    return lrt[t][:, s * DC : (s + 1) * DC]

                for q in range(NQ):
                    o_q = opool.tile([P, 512], f32, name="o_q")
                    for j in range(4):
                        rz = fpool.tile([P, 1], f32, tag="rz", name="rz")
                        nc.vector.reciprocal(
                            rz[:, :], acc_view(q, j)[:, D : D + 1]
                        )
                        nc.vector.scalar_tensor_tensor(
                            o_q[:, j * P : (j + 1) * P],
                            lr_view(q, j)[:, :D],
                            rz[:, :],
                            xn_sb[:, q * 512 + j * P : q * 512 + (j + 1) * P],
                            Alu.mult,
                            Alu.add,
                        )
                    out_view = out[q * 512 : (q + 1) * 512, :].rearrange(
                        "(j p) d -> p j d", p=P
                    )
                    nc.sync.dma_start(
                        out_view, o_q.rearrange("p (j d) -> p j d", j=4)
                    )

    nc.compile()
    return nc


def _prep_inputs(x, W, a1, a2):
    """Per-core packed inputs. Core c: t = c//2, n-half h = c%2.

    xT is host-rotated so the core's own 2048 dest columns come first
    (a rotation does not change a sum over all source nodes).
    """
    import ml_dtypes

    x = np.asarray(x, dtype=np.float32)
    W = np.asarray(W, dtype=np.float32)
    w1 = (W @ np.asarray(a1, np.float32)) * SCALE_INV
    w2 = (W @ np.asarray(a2, np.float32)) * SCALE_INV
    wp = np.concatenate([W, w1[:, None]], axis=1).astype(ml_dtypes.bfloat16)
    w2c = np.ascontiguousarray(w2[:, None])
    in_maps = []
    for c in range(8):
        t, h = c // 2, c % 2
        xt = x[:, t, :].T  # [D, N]
        if h == 1:
            xt = np.concatenate([xt[:, HALF:], xt[:, :HALF]], axis=1)
        xn = x[h * HALF : (h + 1) * HALF, t, :]  # [2048, 128]
        xn_packed = (
            xn.reshape(HALF // P, P, D).transpose(1, 0, 2).reshape(P, HALF)
        )
        in_maps.append(
            {
                "wp": np.ascontiguousarray(wp),
                "w2": w2c,
                "xt": np.ascontiguousarray(xt.astype(ml_dtypes.bfloat16)),
                "xn": np.ascontiguousarray(xn_packed),
            }
        )
    return in_maps


def _run(x, W, a1, a2, trace=False):
    from concourse.bass_utils import run_bass_kernel_spmd

    key = "nc"
    if key not in _CACHE:
        _CACHE[key] = _build()
    nc = _CACHE[key]
    in_maps = _prep_inputs(x, W, a1, a2)
    res = run_bass_kernel_spmd(nc, in_maps, list(range(8)), trace=trace)
    out_full = np.empty((N, T, D), dtype=np.float32)
    for c in range(8):
        t, h = c // 2, c % 2
        out_full[h * HALF : (h + 1) * HALF, t, :] = res.results[c]["out"]
    return out_full, res


def kernel(x, W, a1, a2):
    out, _ = _run(x, W, a1, a2, trace=False)
    return out


# revision 8
# speedup vs baseline: 1.4096x; 1.1602x over previous
"""GAT message-passing kernel for 8 trn2 NeuronCores (v2).

Math (reference):
    Wx = x @ W;  s1 = Wx@a1/sqrt(2D);  s2 = Wx@a2/sqrt(2D)   (per t)
    weight = softmax_m(lrelu(s1[m] + s2[n]));  agg = lrelu(weight @ Wx)
    out = x - agg

Key identities:
  * exp(lrelu(s1+s2)) = max(exp(s1+s2), exp(0.01(s1+s2))); dividing the
    softmax row by exp(s2[n]) (softmax-invariant):
        et_true[m,n] = F1[m] * max(q[m], r[n])
    with q = exp(0.99*s1), F1 = exp(0.01*s1), r = exp(-0.99*s2).
  * F1 folds into the matmul moving operand [F1*Wx | F1], so the score
    tile is ONE single-op DVE tensor_scalar max(r_b, q[m]) (4x_2P mode)
    and the softmax denominator is the accumulated F1 column (exact).
  * out = x - lrelu(num/Z) = x - Lrelu(rz*num) via ACT per-partition scale.

Sharding: 8 cores = 4 t-slices x 2 N-halves; each core aggregates over all
4096 source nodes for its own (t, 2048 dest nodes).

Loop: mt-outer, fully pipelined with the input DMA. Per source tile mt:
projection matmul -> s1 stage -> F1 exp -> evac (F1 fold) -> one
[128,2048] et tile -> 16 accumulation matmuls (4 q x 4 j chunks).
PSUM: 2 banks scratch (proj/r_b) + 6 banks for 16 accumulators packed
three-per-bank.
"""

import sys

if "/opt/trn_rl_repo" not in sys.path:
    sys.path.insert(0, "/opt/trn_rl_repo")

import numpy as np

N, T, D = 4096, 4, 128
P = 128
HALF = N // 2            # 2048 dest nodes per core
MT = N // P              # 32 source tiles
NQ = HALF // 512         # 4 dest chunks of 512
DC = D + 1               # moving operand cols: [F1*Wx | F1]
SCALE_INV = 1.0 / 16.0   # 1/sqrt(2*128)

_CACHE = {}


def _build():
    import concourse.mybir as mybir
    from concourse import bacc
    from concourse.tile import TileContext

    f32 = mybir.dt.float32
    bf16 = mybir.dt.bfloat16
    Alu = mybir.AluOpType
    Act = mybir.ActivationFunctionType

    nc = bacc.Bacc()
    wp = nc.declare_dram_parameter("wp", [P, DC], bf16, isOutput=False)
    w2 = nc.declare_dram_parameter("w2", [P, 1], f32, isOutput=False)
    xt = nc.declare_dram_parameter("xt", [P, N], bf16, isOutput=False)
    xn = nc.declare_dram_parameter("xn", [P, HALF], f32, isOutput=False)
    out = nc.declare_dram_parameter("out", [HALF, D], f32, isOutput=True)

    with TileContext(nc) as tc:
        with (
            tc.tile_pool(name="const", bufs=1) as cpool,
            tc.tile_pool(name="epool", bufs=3) as epool,
            tc.tile_pool(name="fpool", bufs=4) as fpool,
            tc.tile_pool(name="opool", bufs=2) as opool,
        ):
            # ---- input DMAs: params first, then xT quarters, xn last ----
            wp_sb = cpool.tile([P, DC], bf16)
            w2_sb = cpool.tile([P, 1], f32)
            nc.sync.dma_start(wp_sb[:, :], wp[:, :])
            nc.sync.dma_start(w2_sb[:, :], w2[:, :])
            xt_sb = cpool.tile([P, N], bf16)
            for ch in range(4):
                nc.sync.dma_start(
                    xt_sb[:, ch * 1024 : (ch + 1) * 1024],
                    xt[:, ch * 1024 : (ch + 1) * 1024],
                )
            # xn rides the sync queue after xt: a gpsimd-queue DMA would
            # make the entry barrier drain wait for the full transfer.
            xn_sb = cpool.tile([P, HALF], f32)
            nc.sync.dma_start(xn_sb[:, :], xn[:, :])

            # persistent SBUF state
            wxm = cpool.tile([P, MT * DC], bf16)     # [Wx | -1] per mt
            r_b = cpool.tile([P, HALF], bf16)        # exp(-0.99 s2[n]) rows
            q_all = cpool.tile([P, MT], f32)         # exp(0.99 s1)

            with (
                tc.tile_pool(name="spsum", bufs=2, space="PSUM") as spool,
                tc.tile_pool(name="mpsum", bufs=1, space="PSUM") as mpool,
            ):
                # ---- w2b[k,m] = w2[k] broadcast: rank-1 stationary ----
                w2b = cpool.tile([P, P], bf16)
                nc.vector.tensor_scalar(
                    w2b[:, :], wp_sb[:, 0:P], 0.0, w2_sb[:, :],
                    Alu.mult, Alu.add,
                )
                # constant -1 column in every moving-operand slot: the
                # accumulated col 128 is then -Z, so reciprocal gives -1/Z
                # and the finalize is a fused (lr * rz) + xn.
                nc.scalar.activation(
                    wxm.rearrange("p (m c) -> p m c", c=DC)[:, :, D : D + 1],
                    wp_sb[:, 0:MT],
                    Act.Copy, scale=0.0, bias=-1.0,
                )
                # ---- r_b = exp(-0.99 * s2) via replicated-row matmuls ----
                for c in range(4):
                    rb_ps = spool.tile([P, 512], f32, tag="sc", name="rb_ps")
                    nc.tensor.matmul(
                        rb_ps[:, :], w2b[:, :],
                        xt_sb[:, c * 512 : (c + 1) * 512],
                        start=True, stop=True,
                    )
                    nc.scalar.activation(
                        r_b[:, c * 512 : (c + 1) * 512], rb_ps[:, :],
                        Act.Exp, scale=-0.99,
                    )

                # ---- 16 accumulators packed 3-per-bank: 5 trios + 1 ----
                trio = [
                    mpool.tile([P, 3 * DC], f32, tag=f"tr{t}", name=f"tr{t}")
                    for t in range(5)
                ] + [mpool.tile([P, DC], f32, tag="tr5", name="tr5")]

                def acc_view(q, j):
                    idx = q * 4 + j
                    t, s = idx // 3, idx % 3
                    return trio[t][:, s * DC : (s + 1) * DC]

                # ---- main loop: mt-outer, software-pipelined ----
                # proj(mt) is emitted two groups ahead of its matmuls so the
                # proj -> q(ACT) -> et(DVE) chain hides under group mt-2/-1.
                def emit_proj(mt):
                    p_ps = spool.tile([P, 512], f32, tag="sc", name="p_ps")
                    nc.tensor.matmul(
                        p_ps[:, 0:DC],
                        xt_sb[:, mt * P : (mt + 1) * P],
                        wp_sb[:, :],
                        start=True, stop=True,
                    )
                    nc.scalar.activation(
                        q_all[:, mt : mt + 1], p_ps[:, D : D + 1],
                        Act.Exp, scale=0.99,
                    )
                    nc.scalar.activation(
                        wxm[:, mt * DC : mt * DC + D], p_ps[:, 0:D], Act.Copy
                    )

                def emit_et(mt):
                    et = epool.tile([P, HALF], bf16, name="et")
                    nc.vector.tensor_scalar(
                        et[:, :], r_b[:, :], q_all[:, mt : mt + 1],
                        None, Alu.max,
                    )
                    return et

                emit_proj(0)
                emit_proj(1)
                ets = {0: emit_et(0)}
                for mt in range(MT):
                    if mt + 2 < MT:
                        emit_proj(mt + 2)
                    if mt + 1 < MT:
                        ets[mt + 1] = emit_et(mt + 1)
                    et = ets.pop(mt)
                    wv = wxm[:, mt * DC : (mt + 1) * DC]
                    for q in range(NQ):
                        for j in range(4):
                            nc.tensor.matmul(
                                acc_view(q, j),
                                et[:, q * 512 + j * P : q * 512 + (j + 1) * P],
                                wv,
                                start=(mt == 0),
                                stop=(mt == MT - 1),
                            )

                # ---- finalize: lrelu whole trios, then (lr*(-1/Z)) + xn ----
                lrt = [
                    fpool.tile([P, 3 * DC], f32, tag=f"lrt{t}", name=f"lrt{t}")
                    for t in range(5)
                ] + [fpool.tile([P, DC], f32, tag="lrt5", name="lrt5")]
                for t in range(6):
                    nc.scalar.activation(
                        lrt[t][:, :], trio[t][:, :], Act.Lrelu, alpha=0.01
                    )

                def lr_view(q, j):
                    idx = q * 4 + j
                    t, s = idx // 3, idx % 3
                    return lrt[t][:, s * DC : (s + 1) * DC]

                for q in range(NQ):
                    o_q = opool.tile([P, 512], f32, name="o_q")
                    for j in range(4):
                        rz = fpool.tile([P, 1], f32, tag="rz", name="rz")
                        nc.vector.reciprocal(
                            rz[:, :], acc_view(q, j)[:, D : D + 1]
                        )
                        nc.vector.scalar_tensor_tensor(
                            o_q[:, j * P : (j + 1) * P],
                            lr_view(q, j)[:, :D],
                            rz[:, :],
                            xn_sb[:, q * 512 + j * P : q * 512 + (j + 1) * P],
                            Alu.mult,
                            Alu.add,
                        )
                    out_view = out[q * 512 : (q + 1) * 512, :].rearrange(
                        "(j p) d -> p j d", p=P
                    )
                    nc.sync.dma_start(
                        out_view, o_q.rearrange("p (j d) -> p j d", j=4)
                    )

    nc.compile()
    return nc


def _prep_inputs(x, W, a1, a2):
    """Per-core packed inputs. Core c: t = c//2, n-half h = c%2.

    xT is host-rotated so the core's own 2048 dest columns come first
    (a rotation does not change a sum over all source nodes).
    """
    import ml_dtypes

    x = np.asarray(x, dtype=np.float32)
    W = np.asarray(W, dtype=np.float32)
    w1 = (W @ np.asarray(a1, np.float32)) * SCALE_INV
    w2 = (W @ np.asarray(a2, np.float32)) * SCALE_INV
    wp = np.concatenate([W, w1[:, None]], axis=1).astype(ml_dtypes.bfloat16)
    w2c = np.ascontiguousarray(w2[:, None])
    in_maps = []
    for c in range(8):
        t, h = c // 2, c % 2
        xt = x[:, t, :].T  # [D, N]
        if h == 1:
            xt = np.concatenate([xt[:, HALF:], xt[:, :HALF]], axis=1)
        xn = x[h * HALF : (h + 1) * HALF, t, :]  # [2048, 128]
        xn_packed = (
            xn.reshape(HALF // P, P, D).transpose(1, 0, 2).reshape(P, HALF)
        )
        in_maps.append(
            {
                "wp": np.ascontiguousarray(wp),
                "w2": w2c,
                "xt": np.ascontiguousarray(xt.astype(ml_dtypes.bfloat16)),
                "xn": np.ascontiguousarray(xn_packed),
            }
        )
    return in_maps


def _run(x, W, a1, a2, trace=False):
    from concourse.bass_utils import run_bass_kernel_spmd

    key = "nc"
    if key not in _CACHE:
        _CACHE[key] = _build()
    nc = _CACHE[key]
    in_maps = _prep_inputs(x, W, a1, a2)
    res = run_bass_kernel_spmd(nc, in_maps, list(range(8)), trace=trace)
    out_full = np.empty((N, T, D), dtype=np.float32)
    for c in range(8):
        t, h = c // 2, c % 2
        out_full[h * HALF : (h + 1) * HALF, t, :] = res.results[c]["out"]
    return out_full, res


def kernel(x, W, a1, a2):
    out, _ = _run(x, W, a1, a2, trace=False)
    return out


# revision 9
# speedup vs baseline: 1.4399x; 1.0215x over previous
"""GAT message-passing kernel for 8 trn2 NeuronCores (v2).

Math (reference):
    Wx = x @ W;  s1 = Wx@a1/sqrt(2D);  s2 = Wx@a2/sqrt(2D)   (per t)
    weight = softmax_m(lrelu(s1[m] + s2[n]));  agg = lrelu(weight @ Wx)
    out = x - agg

Key identities:
  * exp(lrelu(s1+s2)) = max(exp(s1+s2), exp(0.01(s1+s2))); dividing the
    softmax row by exp(s2[n]) (softmax-invariant):
        et_true[m,n] = F1[m] * max(q[m], r[n])
    with q = exp(0.99*s1), F1 = exp(0.01*s1), r = exp(-0.99*s2).
  * F1 folds into the matmul moving operand [F1*Wx | F1], so the score
    tile is ONE single-op DVE tensor_scalar max(r_b, q[m]) (4x_2P mode)
    and the softmax denominator is the accumulated F1 column (exact).
  * out = x - lrelu(num/Z) = x - Lrelu(rz*num) via ACT per-partition scale.

Sharding: 8 cores = 4 t-slices x 2 N-halves; each core aggregates over all
4096 source nodes for its own (t, 2048 dest nodes).

Loop: mt-outer, fully pipelined with the input DMA. Per source tile mt:
projection matmul -> s1 stage -> F1 exp -> evac (F1 fold) -> one
[128,2048] et tile -> 16 accumulation matmuls (4 q x 4 j chunks).
PSUM: 2 banks scratch (proj/r_b) + 6 banks for 16 accumulators packed
three-per-bank.
"""

import sys

if "/opt/trn_rl_repo" not in sys.path:
    sys.path.insert(0, "/opt/trn_rl_repo")

import numpy as np

N, T, D = 4096, 4, 128
P = 128
HALF = N // 2            # 2048 dest nodes per core
MT = N // P              # 32 source tiles
NQ = HALF // 512         # 4 dest chunks of 512
DC = D + 1               # moving operand cols: [F1*Wx | F1]
SCALE_INV = 1.0 / 16.0   # 1/sqrt(2*128)

_CACHE = {}


def _build():
    import concourse.mybir as mybir
    from concourse import bacc
    from concourse.tile import TileContext

    f32 = mybir.dt.float32
    bf16 = mybir.dt.bfloat16
    Alu = mybir.AluOpType
    Act = mybir.ActivationFunctionType

    nc = bacc.Bacc()
    wp = nc.declare_dram_parameter("wp", [P, DC], bf16, isOutput=False)
    w2 = nc.declare_dram_parameter("w2", [P, 1], f32, isOutput=False)
    xt = nc.declare_dram_parameter("xt", [P, N], bf16, isOutput=False)
    xn = nc.declare_dram_parameter("xn", [P, HALF], f32, isOutput=False)
    out = nc.declare_dram_parameter("out", [HALF, D], f32, isOutput=True)

    with TileContext(nc) as tc:
        with (
            tc.tile_pool(name="const", bufs=1) as cpool,
            tc.tile_pool(name="epool", bufs=3) as epool,
            tc.tile_pool(name="fpool", bufs=4) as fpool,
            tc.tile_pool(name="opool", bufs=2) as opool,
        ):
            # ---- input DMAs: params first, then xT quarters, xn last ----
            wp_sb = cpool.tile([P, DC], bf16)
            w2_sb = cpool.tile([P, 1], f32)
            nc.sync.dma_start(wp_sb[:, :], wp[:, :])
            nc.sync.dma_start(w2_sb[:, :], w2[:, :])
            xt_sb = cpool.tile([P, N], bf16)
            for ch in range(4):
                nc.sync.dma_start(
                    xt_sb[:, ch * 1024 : (ch + 1) * 1024],
                    xt[:, ch * 1024 : (ch + 1) * 1024],
                )
            # xn rides the sync queue after xt: a gpsimd-queue DMA would
            # make the entry barrier drain wait for the full transfer.
            xn_sb = cpool.tile([P, HALF], f32)
            nc.sync.dma_start(xn_sb[:, :], xn[:, :])

            # persistent SBUF state
            wxm = cpool.tile([P, MT * DC], bf16)     # [Wx | -1] per mt
            r_b = cpool.tile([P, HALF], bf16)        # exp(-0.99 s2[n]) rows
            q_all = cpool.tile([P, MT], f32)         # exp(0.99 s1)

            with (
                tc.tile_pool(name="spsum", bufs=2, space="PSUM") as spool,
                tc.tile_pool(name="mpsum", bufs=1, space="PSUM") as mpool,
            ):
                # ---- w2b[k,m] = w2[k] broadcast: rank-1 stationary ----
                w2b = cpool.tile([P, P], bf16)
                nc.vector.tensor_scalar(
                    w2b[:, :], wp_sb[:, 0:P], 0.0, w2_sb[:, :],
                    Alu.mult, Alu.add,
                )
                # constant -1 column in every moving-operand slot: the
                # accumulated col 128 is then -Z, so reciprocal gives -1/Z
                # and the finalize is a fused (lr * rz) + xn.
                nc.scalar.activation(
                    wxm.rearrange("p (m c) -> p m c", c=DC)[:, :, D : D + 1],
                    wp_sb[:, 0:MT],
                    Act.Copy, scale=0.0, bias=-1.0,
                )
                # ---- r_b = exp(-0.99 * s2) via replicated-row matmuls ----
                for c in range(4):
                    rb_ps = spool.tile([P, 512], f32, tag="sc", name="rb_ps")
                    nc.tensor.matmul(
                        rb_ps[:, :], w2b[:, :],
                        xt_sb[:, c * 512 : (c + 1) * 512],
                        start=True, stop=True,
                    )
                    nc.scalar.activation(
                        r_b[:, c * 512 : (c + 1) * 512], rb_ps[:, :],
                        Act.Exp, scale=-0.99,
                    )

                # ---- 16 accumulators packed 3-per-bank: 5 trios + 1 ----
                trio = [
                    mpool.tile([P, 3 * DC], f32, tag=f"tr{t}", name=f"tr{t}")
                    for t in range(5)
                ] + [mpool.tile([P, DC], f32, tag="tr5", name="tr5")]

                def acc_view(q, j):
                    idx = q * 4 + j
                    t, s = idx // 3, idx % 3
                    return trio[t][:, s * DC : (s + 1) * DC]

                # ---- main loop: mt-outer, software-pipelined ----
                # proj(mt) is emitted two groups ahead of its matmuls so the
                # proj -> q(ACT) -> et(DVE) chain hides under group mt-2/-1.
                def emit_proj(mt):
                    p_ps = spool.tile([P, 512], f32, tag="sc", name="p_ps")
                    nc.tensor.matmul(
                        p_ps[:, 0:DC],
                        xt_sb[:, mt * P : (mt + 1) * P],
                        wp_sb[:, :],
                        start=True, stop=True,
                    )
                    nc.scalar.activation(
                        q_all[:, mt : mt + 1], p_ps[:, D : D + 1],
                        Act.Exp, scale=0.99,
                    )
                    nc.scalar.activation(
                        wxm[:, mt * DC : mt * DC + D], p_ps[:, 0:D], Act.Copy
                    )

                def emit_et(mt):
                    et = epool.tile([P, HALF], bf16, name="et")
                    nc.vector.tensor_scalar(
                        et[:, :], r_b[:, :], q_all[:, mt : mt + 1],
                        None, Alu.max,
                    )
                    return et

                emit_proj(0)
                emit_proj(1)
                ets = {0: emit_et(0)}
                for mt in range(MT):
                    if mt + 2 < MT:
                        emit_proj(mt + 2)
                    if mt + 1 < MT:
                        ets[mt + 1] = emit_et(mt + 1)
                    et = ets.pop(mt)
                    wv = wxm[:, mt * DC : (mt + 1) * DC]
                    for q in range(NQ):
                        for j in range(4):
                            nc.tensor.matmul(
                                acc_view(q, j),
                                et[:, q * 512 + j * P : q * 512 + (j + 1) * P],
                                wv,
                                start=(mt == 0),
                                stop=(mt == MT - 1),
                            )

                # ---- finalize: lrelu whole trios, then (lr*(-1/Z)) + xn ----
                lrt = [
                    fpool.tile([P, 3 * DC], f32, tag=f"lrt{t}", name=f"lrt{t}")
                    for t in range(5)
                ] + [fpool.tile([P, DC], f32, tag="lrt5", name="lrt5")]
                for t in range(6):
                    nc.scalar.activation(
                        lrt[t][:, :], trio[t][:, :], Act.Lrelu, alpha=0.01
                    )

                def lr_view(q, j):
                    idx = q * 4 + j
                    t, s = idx // 3, idx % 3
                    return lrt[t][:, s * DC : (s + 1) * DC]

                rzs = fpool.tile([P, 16], f32, tag="rzs", name="rzs")
                for q in range(NQ):
                    for j in range(4):
                        nc.vector.reciprocal(
                            rzs[:, q * 4 + j : q * 4 + j + 1],
                            acc_view(q, j)[:, D : D + 1],
                        )
                for q in range(NQ):
                    o_q = opool.tile([P, 512], f32, name="o_q")
                    for j in range(4):
                        nc.vector.scalar_tensor_tensor(
                            o_q[:, j * P : (j + 1) * P],
                            lr_view(q, j)[:, :D],
                            rzs[:, q * 4 + j : q * 4 + j + 1],
                            xn_sb[:, q * 512 + j * P : q * 512 + (j + 1) * P],
                            Alu.mult,
                            Alu.add,
                        )
                    out_view = out[q * 512 : (q + 1) * 512, :].rearrange(
                        "(j p) d -> p j d", p=P
                    )
                    nc.sync.dma_start(
                        out_view, o_q.rearrange("p (j d) -> p j d", j=4)
                    )

    nc.compile()
    return nc


def _prep_inputs(x, W, a1, a2):
    """Per-core packed inputs. Core c: t = c//2, n-half h = c%2.

    xT is host-rotated so the core's own 2048 dest columns come first
    (a rotation does not change a sum over all source nodes).
    """
    import ml_dtypes

    x = np.asarray(x, dtype=np.float32)
    W = np.asarray(W, dtype=np.float32)
    w1 = (W @ np.asarray(a1, np.float32)) * SCALE_INV
    w2 = (W @ np.asarray(a2, np.float32)) * SCALE_INV
    wp = np.concatenate([W, w1[:, None]], axis=1).astype(ml_dtypes.bfloat16)
    w2c = np.ascontiguousarray(w2[:, None])
    in_maps = []
    for c in range(8):
        t, h = c // 2, c % 2
        xt = x[:, t, :].T  # [D, N]
        if h == 1:
            xt = np.concatenate([xt[:, HALF:], xt[:, :HALF]], axis=1)
        xn = x[h * HALF : (h + 1) * HALF, t, :]  # [2048, 128]
        xn_packed = (
            xn.reshape(HALF // P, P, D).transpose(1, 0, 2).reshape(P, HALF)
        )
        in_maps.append(
            {
                "wp": np.ascontiguousarray(wp),
                "w2": w2c,
                "xt": np.ascontiguousarray(xt.astype(ml_dtypes.bfloat16)),
                "xn": np.ascontiguousarray(xn_packed),
            }
        )
    return in_maps


def _run(x, W, a1, a2, trace=False):
    from concourse.bass_utils import run_bass_kernel_spmd

    key = "nc"
    if key not in _CACHE:
        _CACHE[key] = _build()
    nc = _CACHE[key]
    in_maps = _prep_inputs(x, W, a1, a2)
    res = run_bass_kernel_spmd(nc, in_maps, list(range(8)), trace=trace)
    out_full = np.empty((N, T, D), dtype=np.float32)
    for c in range(8):
        t, h = c // 2, c % 2
        out_full[h * HALF : (h + 1) * HALF, t, :] = res.results[c]["out"]
    return out_full, res


def kernel(x, W, a1, a2):
    out, _ = _run(x, W, a1, a2, trace=False)
    return out


# revision 10
# speedup vs baseline: 1.4446x; 1.0032x over previous
"""GAT message-passing kernel for 8 trn2 NeuronCores (v2).

Math (reference):
    Wx = x @ W;  s1 = Wx@a1/sqrt(2D);  s2 = Wx@a2/sqrt(2D)   (per t)
    weight = softmax_m(lrelu(s1[m] + s2[n]));  agg = lrelu(weight @ Wx)
    out = x - agg

Key identities:
  * exp(lrelu(s1+s2)) = max(exp(s1+s2), exp(0.01(s1+s2))); dividing the
    softmax row by exp(s2[n]) (softmax-invariant):
        et_true[m,n] = F1[m] * max(q[m], r[n])
    with q = exp(0.99*s1), F1 = exp(0.01*s1), r = exp(-0.99*s2).
  * F1 folds into the matmul moving operand [F1*Wx | F1], so the score
    tile is ONE single-op DVE tensor_scalar max(r_b, q[m]) (4x_2P mode)
    and the softmax denominator is the accumulated F1 column (exact).
  * out = x - lrelu(num/Z) = x - Lrelu(rz*num) via ACT per-partition scale.

Sharding: 8 cores = 4 t-slices x 2 N-halves; each core aggregates over all
4096 source nodes for its own (t, 2048 dest nodes).

Loop: mt-outer, fully pipelined with the input DMA. Per source tile mt:
projection matmul -> s1 stage -> F1 exp -> evac (F1 fold) -> one
[128,2048] et tile -> 16 accumulation matmuls (4 q x 4 j chunks).
PSUM: 2 banks scratch (proj/r_b) + 6 banks for 16 accumulators packed
three-per-bank.
"""

import sys

if "/opt/trn_rl_repo" not in sys.path:
    sys.path.insert(0, "/opt/trn_rl_repo")

import numpy as np

N, T, D = 4096, 4, 128
P = 128
HALF = N // 2            # 2048 dest nodes per core
MT = N // P              # 32 source tiles
NQ = HALF // 512         # 4 dest chunks of 512
DC = D + 1               # moving operand cols: [F1*Wx | F1]
SCALE_INV = 1.0 / 16.0   # 1/sqrt(2*128)

_CACHE = {}


def _build():
    import concourse.mybir as mybir
    from concourse import bacc
    from concourse.tile import TileContext

    f32 = mybir.dt.float32
    bf16 = mybir.dt.bfloat16
    Alu = mybir.AluOpType
    Act = mybir.ActivationFunctionType

    nc = bacc.Bacc()
    wp = nc.declare_dram_parameter("wp", [P, DC], bf16, isOutput=False)
    w2 = nc.declare_dram_parameter("w2", [P, 1], f32, isOutput=False)
    xt = nc.declare_dram_parameter("xt", [P, N], bf16, isOutput=False)
    xn = nc.declare_dram_parameter("xn", [P, HALF], f32, isOutput=False)
    out = nc.declare_dram_parameter("out", [HALF, D], f32, isOutput=True)

    with TileContext(nc) as tc:
        with (
            tc.tile_pool(name="const", bufs=1) as cpool,
            tc.tile_pool(name="epool", bufs=3) as epool,
            tc.tile_pool(name="fpool", bufs=4) as fpool,
            tc.tile_pool(name="opool", bufs=2) as opool,
        ):
            # ---- input DMAs: params first, then xT quarters, xn last ----
            wp_sb = cpool.tile([P, DC], bf16)
            w2_sb = cpool.tile([P, 1], f32)
            nc.sync.dma_start(wp_sb[:, :], wp[:, :])
            nc.sync.dma_start(w2_sb[:, :], w2[:, :])
            xt_sb = cpool.tile([P, N], bf16)
            for ch in range(4):
                nc.sync.dma_start(
                    xt_sb[:, ch * 1024 : (ch + 1) * 1024],
                    xt[:, ch * 1024 : (ch + 1) * 1024],
                )
            # xn rides the sync queue after xt: a gpsimd-queue DMA would
            # make the entry barrier drain wait for the full transfer.
            xn_sb = cpool.tile([P, HALF], f32)
            nc.sync.dma_start(xn_sb[:, :], xn[:, :])

            # persistent SBUF state
            wxm = cpool.tile([P, MT * DC], bf16)     # [Wx | -1] per mt
            r_b = cpool.tile([P, HALF], bf16)        # exp(-0.99 s2[n]) rows
            q_all = cpool.tile([P, MT], f32)         # exp(0.99 s1)

            with (
                tc.tile_pool(name="spsum", bufs=2, space="PSUM") as spool,
                tc.tile_pool(name="mpsum", bufs=1, space="PSUM") as mpool,
            ):
                # ---- w2b[k,m] = w2[k] broadcast: rank-1 stationary ----
                w2b = cpool.tile([P, P], bf16)
                nc.vector.tensor_scalar(
                    w2b[:, :], wp_sb[:, 0:P], 0.0, w2_sb[:, :],
                    Alu.mult, Alu.add,
                )
                # constant -1 column in every moving-operand slot: the
                # accumulated col 128 is then -Z, so reciprocal gives -1/Z
                # and the finalize is a fused (lr * rz) + xn.
                nc.scalar.activation(
                    wxm.rearrange("p (m c) -> p m c", c=DC)[:, :, D : D + 1],
                    wp_sb[:, 0:MT],
                    Act.Copy, scale=0.0, bias=-1.0,
                )
                # ---- r_b = exp(-0.99 * s2) via replicated-row matmuls ----
                for c in range(4):
                    rb_ps = spool.tile([P, 512], f32, tag="sc", name="rb_ps")
                    nc.tensor.matmul(
                        rb_ps[:, :], w2b[:, :],
                        xt_sb[:, c * 512 : (c + 1) * 512],
                        start=True, stop=True,
                    )
                    nc.scalar.activation(
                        r_b[:, c * 512 : (c + 1) * 512], rb_ps[:, :],
                        Act.Exp, scale=-0.99,
                    )

                # ---- 16 accumulators packed 3-per-bank: 5 trios + 1 ----
                trio = [
                    mpool.tile([P, 3 * DC], f32, tag=f"tr{t}", name=f"tr{t}")
                    for t in range(5)
                ] + [mpool.tile([P, DC], f32, tag="tr5", name="tr5")]

                def acc_view(q, j):
                    idx = q * 4 + j
                    t, s = idx // 3, idx % 3
                    return trio[t][:, s * DC : (s + 1) * DC]

                # ---- main loop: mt-outer, software-pipelined ----
                # proj(mt) is emitted two groups ahead of its matmuls so the
                # proj -> q(ACT) -> et(DVE) chain hides under group mt-2/-1.
                def emit_proj(mt):
                    p_ps = spool.tile([P, 512], f32, tag="sc", name="p_ps")
                    nc.tensor.matmul(
                        p_ps[:, 0:DC],
                        xt_sb[:, mt * P : (mt + 1) * P],
                        wp_sb[:, :],
                        start=True, stop=True,
                    )
                    nc.scalar.activation(
                        q_all[:, mt : mt + 1], p_ps[:, D : D + 1],
                        Act.Exp, scale=0.99,
                    )
                    nc.scalar.activation(
                        wxm[:, mt * DC : mt * DC + D], p_ps[:, 0:D], Act.Copy
                    )

                def emit_et(mt, h):
                    # half h covers dest q-chunks 2h, 2h+1: only needs the
                    # matching r_b half, so group 0 starts before all of
                    # r_b is ready.
                    et = epool.tile([P, 1024], bf16, name=f"et{h}", tag=f"et{h}")
                    nc.vector.tensor_scalar(
                        et[:, :], r_b[:, h * 1024 : (h + 1) * 1024],
                        q_all[:, mt : mt + 1], None, Alu.max,
                    )
                    return et

                emit_proj(0)
                emit_proj(1)
                ets = {(0, 0): emit_et(0, 0), (0, 1): emit_et(0, 1)}
                for mt in range(MT):
                    if mt + 2 < MT:
                        emit_proj(mt + 2)
                    if mt + 1 < MT:
                        ets[(mt + 1, 0)] = emit_et(mt + 1, 0)
                        ets[(mt + 1, 1)] = emit_et(mt + 1, 1)
                    wv = wxm[:, mt * DC : (mt + 1) * DC]
                    for q in range(NQ):
                        et = ets[(mt, q // 2)]
                        off = (q % 2) * 512
                        for j in range(4):
                            nc.tensor.matmul(
                                acc_view(q, j),
                                et[:, off + j * P : off + (j + 1) * P],
                                wv,
                                start=(mt == 0),
                                stop=(mt == MT - 1),
                            )
                    del ets[(mt, 0)], ets[(mt, 1)]

                # ---- finalize: lrelu whole trios, then (lr*(-1/Z)) + xn ----
                lrt = [
                    fpool.tile([P, 3 * DC], f32, tag=f"lrt{t}", name=f"lrt{t}")
                    for t in range(5)
                ] + [fpool.tile([P, DC], f32, tag="lrt5", name="lrt5")]
                for t in range(6):
                    nc.scalar.activation(
                        lrt[t][:, :], trio[t][:, :], Act.Lrelu, alpha=0.01
                    )

                def lr_view(q, j):
                    idx = q * 4 + j
                    t, s = idx // 3, idx % 3
                    return lrt[t][:, s * DC : (s + 1) * DC]

                rzs = fpool.tile([P, 16], f32, tag="rzs", name="rzs")
                for q in range(NQ):
                    for j in range(4):
                        nc.vector.reciprocal(
                            rzs[:, q * 4 + j : q * 4 + j + 1],
                            acc_view(q, j)[:, D : D + 1],
                        )
                for q in range(NQ):
                    o_q = opool.tile([P, 512], f32, name="o_q")
                    for j in range(4):
                        nc.vector.scalar_tensor_tensor(
                            o_q[:, j * P : (j + 1) * P],
                            lr_view(q, j)[:, :D],
                            rzs[:, q * 4 + j : q * 4 + j + 1],
                            xn_sb[:, q * 512 + j * P : q * 512 + (j + 1) * P],
                            Alu.mult,
                            Alu.add,
                        )
                    out_view = out[q * 512 : (q + 1) * 512, :].rearrange(
                        "(j p) d -> p j d", p=P
                    )
                    nc.sync.dma_start(
                        out_view, o_q.rearrange("p (j d) -> p j d", j=4)
                    )

    nc.compile()
    return nc


def _prep_inputs(x, W, a1, a2):
    """Per-core packed inputs. Core c: t = c//2, n-half h = c%2.

    xT is host-rotated so the core's own 2048 dest columns come first
    (a rotation does not change a sum over all source nodes).
    """
    import ml_dtypes

    x = np.asarray(x, dtype=np.float32)
    W = np.asarray(W, dtype=np.float32)
    w1 = (W @ np.asarray(a1, np.float32)) * SCALE_INV
    w2 = (W @ np.asarray(a2, np.float32)) * SCALE_INV
    wp = np.concatenate([W, w1[:, None]], axis=1).astype(ml_dtypes.bfloat16)
    w2c = np.ascontiguousarray(w2[:, None])
    in_maps = []
    for c in range(8):
        t, h = c // 2, c % 2
        xt = x[:, t, :].T  # [D, N]
        if h == 1:
            xt = np.concatenate([xt[:, HALF:], xt[:, :HALF]], axis=1)
        xn = x[h * HALF : (h + 1) * HALF, t, :]  # [2048, 128]
        xn_packed = (
            xn.reshape(HALF // P, P, D).transpose(1, 0, 2).reshape(P, HALF)
        )
        in_maps.append(
            {
                "wp": np.ascontiguousarray(wp),
                "w2": w2c,
                "xt": np.ascontiguousarray(xt.astype(ml_dtypes.bfloat16)),
                "xn": np.ascontiguousarray(xn_packed),
            }
        )
    return in_maps


def _run(x, W, a1, a2, trace=False):
    from concourse.bass_utils import run_bass_kernel_spmd

    key = "nc"
    if key not in _CACHE:
        _CACHE[key] = _build()
    nc = _CACHE[key]
    in_maps = _prep_inputs(x, W, a1, a2)
    res = run_bass_kernel_spmd(nc, in_maps, list(range(8)), trace=trace)
    out_full = np.empty((N, T, D), dtype=np.float32)
    for c in range(8):
        t, h = c // 2, c % 2
        out_full[h * HALF : (h + 1) * HALF, t, :] = res.results[c]["out"]
    return out_full, res


def kernel(x, W, a1, a2):
    out, _ = _run(x, W, a1, a2, trace=False)
    return out


# revision 15
# speedup vs baseline: 1.4462x; 1.0011x over previous
"""GAT message-passing kernel for 8 trn2 NeuronCores (v2).

Math (reference):
    Wx = x @ W;  s1 = Wx@a1/sqrt(2D);  s2 = Wx@a2/sqrt(2D)   (per t)
    weight = softmax_m(lrelu(s1[m] + s2[n]));  agg = lrelu(weight @ Wx)
    out = x - agg

Key identities:
  * exp(lrelu(s1+s2)) = max(exp(s1+s2), exp(0.01(s1+s2))); dividing the
    softmax row by exp(s2[n]) (softmax-invariant):
        et_true[m,n] = F1[m] * max(q[m], r[n])
    with q = exp(0.99*s1), F1 = exp(0.01*s1), r = exp(-0.99*s2).
  * F1 folds into the matmul moving operand [F1*Wx | F1], so the score
    tile is ONE single-op DVE tensor_scalar max(r_b, q[m]) (4x_2P mode)
    and the softmax denominator is the accumulated F1 column (exact).
  * out = x - lrelu(num/Z) = x - Lrelu(rz*num) via ACT per-partition scale.

Sharding: 8 cores = 4 t-slices x 2 N-halves; each core aggregates over all
4096 source nodes for its own (t, 2048 dest nodes).

Loop: mt-outer, fully pipelined with the input DMA. Per source tile mt:
projection matmul -> s1 stage -> F1 exp -> evac (F1 fold) -> one
[128,2048] et tile -> 16 accumulation matmuls (4 q x 4 j chunks).
PSUM: 2 banks scratch (proj/r_b) + 6 banks for 16 accumulators packed
three-per-bank.
"""

import sys

if "/opt/trn_rl_repo" not in sys.path:
    sys.path.insert(0, "/opt/trn_rl_repo")

import numpy as np

N, T, D = 4096, 4, 128
P = 128
HALF = N // 2            # 2048 dest nodes per core
MT = N // P              # 32 source tiles
NQ = HALF // 512         # 4 dest chunks of 512
DC = D + 1               # moving operand cols: [F1*Wx | F1]
SCALE_INV = 1.0 / 16.0   # 1/sqrt(2*128)

_CACHE = {}


def _build():
    import concourse.mybir as mybir
    from concourse import bacc
    from concourse.tile import TileContext

    f32 = mybir.dt.float32
    bf16 = mybir.dt.bfloat16
    Alu = mybir.AluOpType
    Act = mybir.ActivationFunctionType

    nc = bacc.Bacc()
    # wp packs [W | w1 | w2-as-2-bf16-bitcast]: one DMA for all params
    wp = nc.declare_dram_parameter("wp", [P, DC + 3], bf16, isOutput=False)
    xt = nc.declare_dram_parameter("xt", [P, N], bf16, isOutput=False)
    xn = nc.declare_dram_parameter("xn", [P, HALF], f32, isOutput=False)
    out = nc.declare_dram_parameter("out", [HALF, D], f32, isOutput=True)

    with TileContext(nc) as tc:
        with (
            tc.tile_pool(name="const", bufs=1) as cpool,
            tc.tile_pool(name="epool", bufs=3) as epool,
            tc.tile_pool(name="fpool", bufs=4) as fpool,
            tc.tile_pool(name="opool", bufs=2) as opool,
        ):
            # ---- input DMAs: params first, then xT halves, xn last ----
            wpx_sb = cpool.tile([P, DC + 3], bf16)
            nc.sync.dma_start(wpx_sb[:, :], wp[:, :])
            wp_sb = wpx_sb[:, 0:DC]
            w2_sb = wpx_sb.bitcast(f32)[:, (DC + 1) // 2 : (DC + 1) // 2 + 1]
            xt_sb = cpool.tile([P, N], bf16)
            for ch in range(2):
                nc.sync.dma_start(
                    xt_sb[:, ch * 2048 : (ch + 1) * 2048],
                    xt[:, ch * 2048 : (ch + 1) * 2048],
                )
            # xn rides the sync queue after xt: a gpsimd-queue DMA would
            # make the entry barrier drain wait for the full transfer.
            xn_sb = cpool.tile([P, HALF], f32)
            nc.sync.dma_start(xn_sb[:, :], xn[:, :])

            # persistent SBUF state
            wxm = cpool.tile([P, MT * DC], bf16)     # [Wx | -1] per mt
            r_b = cpool.tile([P, HALF], bf16)        # exp(-0.99 s2[n]) rows
            q_all = cpool.tile([P, MT], f32)         # exp(0.99 s1)

            with (
                tc.tile_pool(name="spsum", bufs=2, space="PSUM") as spool,
                tc.tile_pool(name="mpsum", bufs=1, space="PSUM") as mpool,
            ):
                # ---- w2b[k,m] = w2[k] broadcast: rank-1 stationary ----
                w2b = cpool.tile([P, P], bf16)
                nc.vector.tensor_scalar(
                    w2b[:, :], wp_sb[:, 0:P], 0.0, w2_sb[:, :],
                    Alu.mult, Alu.add,
                )
                # constant -1 column in every moving-operand slot: the
                # accumulated col 128 is then -Z, so reciprocal gives -1/Z
                # and the finalize is a fused (lr * rz) + xn.
                nc.scalar.activation(
                    wxm.rearrange("p (m c) -> p m c", c=DC)[:, :, D : D + 1],
                    wp_sb[:, 0:MT],
                    Act.Copy, scale=0.0, bias=-1.0,
                )
                # ---- r_b = exp(-0.99 * s2) via replicated-row matmuls ----
                for c in range(4):
                    rb_ps = spool.tile([P, 512], f32, tag="sc", name="rb_ps")
                    nc.tensor.matmul(
                        rb_ps[:, :], w2b[:, :],
                        xt_sb[:, c * 512 : (c + 1) * 512],
                        start=True, stop=True,
                    )
                    nc.scalar.activation(
                        r_b[:, c * 512 : (c + 1) * 512], rb_ps[:, :],
                        Act.Exp, scale=-0.99,
                    )

                # ---- 16 accumulators packed 3-per-bank: 5 trios + 1 ----
                trio = [
                    mpool.tile([P, 3 * DC], f32, tag=f"tr{t}", name=f"tr{t}")
                    for t in range(5)
                ] + [mpool.tile([P, DC], f32, tag="tr5", name="tr5")]

                def acc_view(q, j):
                    idx = q * 4 + j
                    t, s = idx // 3, idx % 3
                    return trio[t][:, s * DC : (s + 1) * DC]

                # ---- main loop: mt-outer, software-pipelined ----
                # proj(mt) is emitted two groups ahead of its matmuls so the
                # proj -> q(ACT) -> et(DVE) chain hides under group mt-2/-1.
                def emit_proj(mt):
                    p_ps = spool.tile([P, 512], f32, tag="sc", name="p_ps")
                    nc.tensor.matmul(
                        p_ps[:, 0:DC],
                        xt_sb[:, mt * P : (mt + 1) * P],
                        wp_sb[:, :],
                        start=True, stop=True,
                    )
                    nc.scalar.activation(
                        q_all[:, mt : mt + 1], p_ps[:, D : D + 1],
                        Act.Exp, scale=0.99,
                    )
                    nc.scalar.activation(
                        wxm[:, mt * DC : mt * DC + D], p_ps[:, 0:D], Act.Copy
                    )

                def emit_et(mt, h):
                    # half h covers dest q-chunks 2h, 2h+1: only needs the
                    # matching r_b half, so group 0 starts before all of
                    # r_b is ready.
                    et = epool.tile([P, 1024], bf16, name=f"et{h}", tag=f"et{h}")
                    nc.vector.tensor_scalar(
                        et[:, :], r_b[:, h * 1024 : (h + 1) * 1024],
                        q_all[:, mt : mt + 1], None, Alu.max,
                    )
                    return et

                emit_proj(0)
                emit_proj(1)
                ets = {(0, 0): emit_et(0, 0), (0, 1): emit_et(0, 1)}
                for mt in range(MT):
                    if mt + 2 < MT:
                        emit_proj(mt + 2)
                    if mt + 1 < MT:
                        ets[(mt + 1, 0)] = emit_et(mt + 1, 0)
                        ets[(mt + 1, 1)] = emit_et(mt + 1, 1)
                    wv = wxm[:, mt * DC : (mt + 1) * DC]
                    for q in range(NQ):
                        et = ets[(mt, q // 2)]
                        off = (q % 2) * 512
                        for j in range(4):
                            nc.tensor.matmul(
                                acc_view(q, j),
                                et[:, off + j * P : off + (j + 1) * P],
                                wv,
                                start=(mt == 0),
                                stop=(mt == MT - 1),
                            )
                    del ets[(mt, 0)], ets[(mt, 1)]

                # ---- finalize: lrelu whole trios, then (lr*(-1/Z)) + xn ----
                lrt = [
                    fpool.tile([P, 3 * DC], f32, tag=f"lrt{t}", name=f"lrt{t}")
                    for t in range(5)
                ] + [fpool.tile([P, DC], f32, tag="lrt5", name="lrt5")]
                for t in range(6):
                    nc.scalar.activation(
                        lrt[t][:, :], trio[t][:, :], Act.Lrelu, alpha=0.01
                    )

                def lr_view(q, j):
                    idx = q * 4 + j
                    t, s = idx // 3, idx % 3
                    return lrt[t][:, s * DC : (s + 1) * DC]

                rzs = fpool.tile([P, 16], f32, tag="rzs", name="rzs")
                for q in range(NQ):
                    for j in range(4):
                        nc.vector.reciprocal(
                            rzs[:, q * 4 + j : q * 4 + j + 1],
                            acc_view(q, j)[:, D : D + 1],
                        )
                for q in range(NQ):
                    o_q = opool.tile([P, 512], f32, name="o_q")
                    for j in range(4):
                        nc.vector.scalar_tensor_tensor(
                            o_q[:, j * P : (j + 1) * P],
                            lr_view(q, j)[:, :D],
                            rzs[:, q * 4 + j : q * 4 + j + 1],
                            xn_sb[:, q * 512 + j * P : q * 512 + (j + 1) * P],
                            Alu.mult,
                            Alu.add,
                        )
                    out_view = out[q * 512 : (q + 1) * 512, :].rearrange(
                        "(j p) d -> p j d", p=P
                    )
                    nc.sync.dma_start(
                        out_view, o_q.rearrange("p (j d) -> p j d", j=4)
                    )

    nc.compile()
    return nc


def _prep_inputs(x, W, a1, a2):
    """Per-core packed inputs. Core c: t = c//2, n-half h = c%2.

    xT is host-rotated so the core's own 2048 dest columns come first
    (a rotation does not change a sum over all source nodes).
    """
    import ml_dtypes

    x = np.asarray(x, dtype=np.float32)
    W = np.asarray(W, dtype=np.float32)
    w1 = (W @ np.asarray(a1, np.float32)) * SCALE_INV
    w2 = (W @ np.asarray(a2, np.float32)) * SCALE_INV
    wp_bf = np.concatenate([W, w1[:, None]], axis=1).astype(ml_dtypes.bfloat16)
    w2_pair = np.ascontiguousarray(w2[:, None].astype(np.float32)).view(
        ml_dtypes.bfloat16
    )
    pad = np.zeros((P, 2), dtype=ml_dtypes.bfloat16)
    wpx = np.ascontiguousarray(
        np.concatenate([wp_bf, pad[:, :1], w2_pair], axis=1)
    )
    in_maps = []
    for c in range(8):
        t, h = c // 2, c % 2
        xt = x[:, t, :].T  # [D, N]
        if h == 1:
            xt = np.concatenate([xt[:, HALF:], xt[:, :HALF]], axis=1)
        xn = x[h * HALF : (h + 1) * HALF, t, :]  # [2048, 128]
        xn_packed = (
            xn.reshape(HALF // P, P, D).transpose(1, 0, 2).reshape(P, HALF)
        )
        in_maps.append(
            {
                "wp": wpx,
                "xt": np.ascontiguousarray(xt.astype(ml_dtypes.bfloat16)),
                "xn": np.ascontiguousarray(xn_packed),
            }
        )
    return in_maps


def _run(x, W, a1, a2, trace=False):
    from concourse.bass_utils import run_bass_kernel_spmd

    key = "nc"
    if key not in _CACHE:
        _CACHE[key] = _build()
    nc = _CACHE[key]
    in_maps = _prep_inputs(x, W, a1, a2)
    res = run_bass_kernel_spmd(nc, in_maps, list(range(8)), trace=trace)
    out_full = np.empty((N, T, D), dtype=np.float32)
    for c in range(8):
        t, h = c // 2, c % 2
        out_full[h * HALF : (h + 1) * HALF, t, :] = res.results[c]["out"]
    return out_full, res


def kernel(x, W, a1, a2):
    out, _ = _run(x, W, a1, a2, trace=False)
    return out
